# revision 6
# baseline (speedup 1.0000x reference)
"""Grouped BERT self-attention on 8 TRN2 NeuronCores.

Problem: G=4 groups, B=4 batch, L=512 seq, C=768 (12 heads x 64).
Sharding: the 16 (g, b) attention problems are embarrassingly parallel;
each core handles one group g = core//2 and two batches. Weights are
per-group so each core loads exactly one group's weights. No collectives.

Per-(g,b) on-chip dataflow (bf16 matmul inputs, fp32 accumulation):
  qT[d,l] = Wq[c,d].T @ hsT[c,l]    (weights in natural layout = lhsT;
                                     bias folded into PSUM->SBUF copy)
  kT[d,l] = Wk[c,d].T @ hsT[c,l]
  v[m,d]  = hsT[c,m].T @ Wv[c,d]    (+bias, stored [m, head, 65] with a
                                     ones column per head -> softmax denom)
  ST[m,l] = kT[d,m].T @ qT[d,l]     (heads paired on partitions 0:64/64:128
                                     -> concurrent PE row-tiles, shared
                                     2-bank PSUM tile)
  E[m,l]  = exp(0.125*ST + mask[m]) (one ACT op per head-pair, bf16 out)
  ctxT[d+1,l] = v_aug[m,d+1].T @ E[m,l]   (accumulate over m chunks;
                                     row d=64 is the softmax denominator)
  ctx[l,d]    = DMA-transpose(ctxT) (bf16, padded to 80 partitions)
  out[l,:] = ctx * (1/denom)        (batched: one reciprocal + one
                                     broadcast multiply per 128 tokens)

PE emission order pipelines (qk chunk -> scores -> exp) one step ahead
of PV so the TensorEngine never waits on the ScalarEngine's exp.
"""

import numpy as np
import ml_dtypes

import concourse.bacc as bacc
import concourse.bass as bass
import concourse.tile as tile
import concourse.mybir as mybir
from concourse import bass_utils

# avoid FishPath artifact upload in the axon trace path
bass_utils.upload_artifacts = lambda tmpdir: tmpdir

G, B, L, C = 4, 4, 512, 768
NH, DH = 12, 64
NB = 2          # batches per core
CCH = C // 128  # 6 contraction chunks
LCH = L // 128  # 4 seq chunks
N_CORES = 8
PAD = 80        # ctxT rows padded to a multiple of 16 for DMA transpose

BF16 = mybir.dt.bfloat16
F32 = mybir.dt.float32
NPBF16 = ml_dtypes.bfloat16

_COMPILED = None


def _build():
    nc = bacc.Bacc("TRN2", target_bir_lowering=False, debug=False)
    AF = mybir.ActivationFunctionType

    hst_d = nc.declare_dram_parameter("hst", [NB, 128, CCH, L], BF16, isOutput=False)
    wq_d = nc.declare_dram_parameter("wq", [128, CCH, C], BF16, isOutput=False)
    wk_d = nc.declare_dram_parameter("wk", [128, CCH, C], BF16, isOutput=False)
    wv_d = nc.declare_dram_parameter("wv", [128, CCH, C], BF16, isOutput=False)
    bq_d = nc.declare_dram_parameter("bq", [128, CCH], F32, isOutput=False)
    bk_d = nc.declare_dram_parameter("bk", [128, CCH], F32, isOutput=False)
    bvb_d = nc.declare_dram_parameter("bvb", [128, C], BF16, isOutput=False)
    mask_d = nc.declare_dram_parameter("mask", [NB, 128, LCH], F32, isOutput=False)
    out_d = nc.declare_dram_parameter("out", [NB, LCH, 128, C], F32, isOutput=True)

    with tile.TileContext(nc) as tc:
        with (
            tc.tile_pool(name="wpool", bufs=1) as wpool,
            tc.tile_pool(name="hpool", bufs=2) as hpool,
            tc.tile_pool(name="qkpool", bufs=2) as qkpool,
            tc.tile_pool(name="vpool", bufs=2 * LCH) as vpool,
            tc.tile_pool(name="epool", bufs=12) as epool,
            tc.tile_pool(name="tpool", bufs=4) as tpool,
            tc.tile_pool(name="cpool", bufs=2 * LCH) as cpool,
            tc.tile_pool(name="rpool", bufs=4) as rpool,
            tc.tile_pool(name="pqk", bufs=2, space=bass.MemorySpace.PSUM) as pqk,
            tc.tile_pool(name="pss", bufs=2, space=bass.MemorySpace.PSUM) as pss_pool,
            tc.tile_pool(name="ppv", bufs=2, space=bass.MemorySpace.PSUM) as ppv,
        ):
            # ---- persistent constants ----
            wq = wpool.tile([128, CCH, C], BF16, tag="wq")
            wk = wpool.tile([128, CCH, C], BF16, tag="wk")
            wv = wpool.tile([128, CCH, C], BF16, tag="wv")
            bq = wpool.tile([128, CCH], F32, tag="bq")
            bk = wpool.tile([128, CCH], F32, tag="bk")
            bvb = wpool.tile([128, C], BF16, tag="bvb")
            for j in range(CCH):  # chunked so the first matmuls start early
                nc.sync.dma_start(wq[:, j], wq_d[:, j])
                nc.sync.dma_start(wk[:, j], wk_d[:, j])
                nc.sync.dma_start(wv[:, j], wv_d[:, j])
            nc.sync.dma_start(bq[:], bq_d[:])
            nc.sync.dma_start(bk[:], bk_d[:])
            nc.sync.dma_start(bvb[:], bvb_d[:])

            hst, msk, qt, kt, vt, ctxs, e_of = {}, {}, {}, {}, {}, {}, {}

            def emit_load(b):
                hst[b] = hpool.tile([128, CCH, L], BF16, tag="hst", name=f"hst{b}")
                msk[b] = hpool.tile([128, LCH], F32, tag="mask", name=f"msk{b}")
                for j in range(CCH):
                    nc.sync.dma_start(hst[b][:, j], hst_d[b, :, j])
                nc.sync.dma_start(msk[b][:], mask_d[b])

            def emit_v(b):
                vt[b] = [
                    vpool.tile([128, NH, DH + 1], BF16, tag="v", name=f"v{b}_{t}")
                    for t in range(LCH)
                ]
                for t in range(LCH):
                    for half in range(2):
                        ncol = C // 2  # 384
                        ps = pqk.tile([128, ncol], F32, tag="big", name="psv")
                        for k in range(CCH):
                            nc.tensor.matmul(
                                ps[:],
                                hst[b][:, k, 128 * t : 128 * (t + 1)],
                                wv[:, k, half * ncol : (half + 1) * ncol],
                                start=(k == 0),
                                stop=(k == CCH - 1),
                            )
                        nh2 = NH // 2
                        nc.vector.tensor_add(
                            vt[b][t][:, half * nh2 : (half + 1) * nh2, 0:DH],
                            ps[:].rearrange("p (h d) -> p h d", d=DH),
                            bvb[:, half * ncol : (half + 1) * ncol].rearrange(
                                "p (h d) -> p h d", d=DH
                            ),
                        )
                    nc.vector.memset(vt[b][t][:, :, DH : DH + 1], 1.0)

            def emit_qk_chunk(b, j):
                if j == 0:
                    qt[b] = qkpool.tile(
                        [128, CCH, L], BF16, tag="qt", name=f"qt{b}"
                    )
                    kt[b] = qkpool.tile(
                        [128, CCH, L], BF16, tag="kt", name=f"kt{b}"
                    )
                for w, bias, dst in ((wq, bq, qt[b]), (wk, bk, kt[b])):
                    ps = pqk.tile([128, L], F32, tag="big", name="psqk")
                    for k in range(CCH):
                        nc.tensor.matmul(
                            ps[:],
                            w[:, k, 128 * j : 128 * (j + 1)],
                            hst[b][:, k, :],
                            start=(k == 0),
                            stop=(k == CCH - 1),
                        )
                    nc.vector.tensor_scalar_add(
                        dst[:, j, :], ps[:], bias[:, j : j + 1]
                    )

            def emit_scores(b, hp):
                e = [
                    epool.tile([128, 2, L], BF16, tag="e", name=f"e{b}_{hp}_{mc}")
                    for mc in range(LCH)
                ]
                e_of[(b, hp)] = e
                for mc in range(LCH):
                    ps = pss_pool.tile([128, 2, L], F32, tag="pss", name="pss")
                    for h2 in range(2):
                        pr = slice(64 * h2, 64 * (h2 + 1))
                        nc.tensor.matmul(
                            ps[:, h2, :],
                            kt[b][pr, hp, 128 * mc : 128 * (mc + 1)],
                            qt[b][pr, hp, :],
                        )
                    nc.scalar.activation(
                        e[mc][:], ps[:], AF.Exp,
                        bias=msk[b][:, mc : mc + 1], scale=0.125,
                    )

            def emit_pv(b, hp):
                e = e_of.pop((b, hp))
                for h2 in range(2):
                    head = 2 * hp + h2
                    pc = ppv.tile([DH + 1, L], F32, tag="pv", name="pc")
                    for mc in range(LCH):
                        nc.tensor.matmul(
                            pc[:],
                            vt[b][mc][:, head, :],
                            e[mc][:, h2, :],
                            start=(mc == 0),
                            stop=(mc == LCH - 1),
                        )
                    ct = tpool.tile([PAD, L], BF16, tag="ct", name="ct")
                    nc.vector.tensor_copy(ct[0 : DH + 1, :], pc[:])
                    for lc in range(LCH):
                        nc.sync.dma_start_transpose(
                            out=ctxs[b][lc][:, head, :],
                            in_=ct[:, 128 * lc : 128 * (lc + 1)],
                        )

            def emit_ctx_alloc(b):
                ctxs[b] = [
                    cpool.tile([128, NH, PAD], BF16, tag="ctt", name=f"ctt{b}_{lc}")
                    for lc in range(LCH)
                ]

            def emit_finish(b):
                for lc in range(LCH):
                    rec = rpool.tile([128, NH, 1], F32, tag="rec", name="rec")
                    nc.vector.reciprocal(
                        rec[:], ctxs[b][lc][:, :, DH : DH + 1]
                    )
                    ctx = cpool.tile([128, NH, DH], F32, tag="ctx", name="ctx")
                    nc.vector.tensor_mul(
                        ctx[:],
                        ctxs[b][lc][:, :, 0:DH],
                        rec[:].broadcast_to((128, NH, DH)),
                    )
                    nc.sync.dma_start(
                        out_d[b, lc], ctx[:].rearrange("p h d -> p (h d)")
                    )

            # ---- emission schedule: PV lags (scores, exp) by one unit ----
            emit_load(0)
            emit_load(1)
            emit_ctx_alloc(0)
            emit_ctx_alloc(1)
            emit_v(0)
            pv_q = []

            def pop_pv():
                bb, hh = pv_q.pop(0)
                emit_pv(bb, hh)
                if hh == CCH - 1:
                    emit_finish(bb)

            for b in range(NB):
                if b == 1:
                    emit_v(1)
                for hp in range(CCH):
                    emit_qk_chunk(b, hp)
                    emit_scores(b, hp)
                    pv_q.append((b, hp))
                    if len(pv_q) >= 2:
                        pop_pv()
            while pv_q:
                pop_pv()

    nc.compile()
    return nc


def _get_compiled():
    global _COMPILED
    if _COMPILED is None:
        _COMPILED = _build()
    return _COMPILED


def _prep_core(hs, mask, wq, wk, wv, bq, bk, bv, g, b0):
    hs_gb = np.ascontiguousarray(hs[g, b0 : b0 + NB])  # [2, L, C]
    # hst[b, p, j, l] = hs[g, b0+b, l, 128j+p]
    hst = np.ascontiguousarray(
        hs_gb.transpose(0, 2, 1).reshape(NB, CCH, 128, L).transpose(0, 2, 1, 3)
    ).astype(NPBF16)

    def wprep(w):
        # [p, j, d] = W[128j+p, d]
        return np.ascontiguousarray(
            w[g].reshape(CCH, 128, C).transpose(1, 0, 2)
        ).astype(NPBF16)

    bq_t = np.ascontiguousarray(bq[g, 0].reshape(CCH, 128).T).astype(np.float32)
    bk_t = np.ascontiguousarray(bk[g, 0].reshape(CCH, 128).T).astype(np.float32)
    bvb = np.ascontiguousarray(
        np.broadcast_to(bv[g, 0], (128, C))
    ).astype(NPBF16)
    # mask[b, p, mc] = mask[g, b0+b, 0, 0, 128mc+p]
    msk = np.ascontiguousarray(
        mask[g, b0 : b0 + NB, 0, 0].reshape(NB, LCH, 128).transpose(0, 2, 1)
    ).astype(np.float32)
    return {
        "hst": hst,
        "wq": wprep(wq),
        "wk": wprep(wk),
        "wv": wprep(wv),
        "bq": bq_t,
        "bk": bk_t,
        "bvb": bvb,
        "mask": msk,
    }


def kernel(
    hidden_states,
    attention_mask,
    query_weight,
    query_bias,
    key_weight,
    key_bias,
    value_weight,
    value_bias,
    _trace=False,
):
    hs = np.asarray(hidden_states, dtype=np.float32)
    mask = np.asarray(attention_mask, dtype=np.float32)
    wq = np.asarray(query_weight, dtype=np.float32)
    wk = np.asarray(key_weight, dtype=np.float32)
    wv = np.asarray(value_weight, dtype=np.float32)
    bq = np.asarray(query_bias, dtype=np.float32)
    bk = np.asarray(key_bias, dtype=np.float32)
    bv = np.asarray(value_bias, dtype=np.float32)

    nc = _get_compiled()
    in_maps = []
    for c in range(N_CORES):
        g, b0 = c // 2, NB * (c % 2)
        in_maps.append(_prep_core(hs, mask, wq, wk, wv, bq, bk, bv, g, b0))

    res = bass_utils.run_bass_kernel_spmd(
        nc, in_maps, core_ids=list(range(N_CORES)), trace=_trace
    )

    out = np.empty((G, B, L, C), dtype=np.float32)
    for c in range(N_CORES):
        g, b0 = c // 2, NB * (c % 2)
        o = res.results[c]["out"]  # [NB, LCH, 128, C]
        out[g, b0 : b0 + NB] = o.reshape(NB, L, C)
    if _trace:
        kernel.last_exec_time_ns = res.exec_time_ns
    return out
